# revision 27
# baseline (speedup 1.0000x reference)
"""Multi-head attention (B=4, S=2048, D=1024, H=16, Dk=64) on 8 trn2 NeuronCores.

Sharding: core = (batch b, head-group g), b in 0..3, g in 0..1.  Each core
computes attention for its batch and its 8 heads plus the partial out
projection for its 512 rows of Wo; host sums the two partials per batch and
adds bo.

Key structural ideas (vs the 922us baseline):
  * Host-side prep: q/k/v are transposed to [D, S] on the host, so the device
    does no PE transposes at all.  k/v are additionally COMPACTED: only the
    ~1024 unmasked key rows (mask==1) are sent, padded to SP=1152 (9 chunks of
    128 instead of 16) - the mask zeroes ~44% of the attention work, so skip
    it.  Padding slots get a -1e9 bias in the exp (probs underflow to 0).
  * bf16 everywhere on the PE (f32r runs ~1.5ns/row with 300ns serial weight
    loads on real hw; bf16 streams 1 col/cycle with hidden loads); PSUM
    accumulates f32.  khT/qhT are stored per-head as [128, head, s] with rows
    64..127 zeroed so every matmul in the program uses the same 128x128 PE
    tile configuration - mixing tile configs serializes the PE pipeline.
  * No K=1 bias matmuls: q/k biases ride along the PSUM->SBUF copies
    (per-partition bias on ACT/DVE); the v bias is folded out exactly
    (softmax rows sum to 1, so bv shifts attn by bv: the host adds bv @ Wo
    to the output bias instead).
  * Phase B is software-pipelined per 1024-query block and head: per sk step
    the PE emission order is [scores(sk) x2, attn-half1(prev head, sk),
    attn-half0(sk)] so the PE never sits behind the ACT exp (the pacer at
    ~1.1us per 128x1024 exp); the attn matmul is split into two 512-wide
    halves accumulated in different passes so the at tiles are single-bank,
    leaving one spare PSUM bank for filler work.
  * Filler work keeps the PE busy through B's exp-paced stretch: the q
    projection for the second query block runs inside phase B of the first
    block, and the out projection of block 0 runs inside phase B of block 1.
  * Few, large, strided input DMAs (one per tensor): many small dma_starts
    push Tile's semaphore-wrap barriers into the middle of the input stream,
    which can serialize the whole kernel behind phase-B progress.
  * Softmax denominator comes free from a ones column appended to vh (row 64
    of the attn PSUM); normalization = DVE reciprocal + gpsimd
    partition_broadcast + DVE multiply (no PE replicate matmuls, no PSUM).
"""

import sys

sys.path.insert(0, "/opt/trn_rl_repo")

import numpy as np

B, S, D, H, DK = 4, 2048, 1024, 16, 64
CPG = 512          # projection columns per core (8 heads x 64)
SP = 1152          # compacted+padded key count (9 chunks of 128)
NSK = SP // 128    # 9
NJ = D // 128      # 8 contraction chunks
NCORES = 8

_cache = {}


def _build_nc():
    import concourse.bass as bass
    import concourse.tile as tile
    from concourse import bacc, mybir

    f32 = mybir.dt.float32
    R = mybir.dt.float32r
    BF = mybir.dt.bfloat16
    Exp = mybir.ActivationFunctionType.Exp

    nc = bacc.Bacc("TRN2", target_bir_lowering=False, debug=False)

    qT_d = nc.dram_tensor("qT", [D, S], BF, kind="ExternalInput").ap()
    kT_d = nc.dram_tensor("kT", [D, SP], BF, kind="ExternalInput").ap()
    vT_d = nc.dram_tensor("vT", [D, SP], BF, kind="ExternalInput").ap()
    wq_d = nc.dram_tensor("wq", [D, CPG], BF, kind="ExternalInput").ap()
    wk_d = nc.dram_tensor("wk", [D, CPG], BF, kind="ExternalInput").ap()
    wv_d = nc.dram_tensor("wv", [D, CPG], BF, kind="ExternalInput").ap()
    wo_d = nc.dram_tensor("wo", [CPG, D], BF, kind="ExternalInput").ap()
    bqT_d = nc.dram_tensor("bqT", [128, 4], f32, kind="ExternalInput").ap()
    bkT_d = nc.dram_tensor("bkT", [128, 4], f32, kind="ExternalInput").ap()
    mb_d = nc.dram_tensor("maskbias", [128, NSK], f32, kind="ExternalInput").ap()
    ones_d = nc.dram_tensor("ones", [128, 512], BF, kind="ExternalInput").ap()
    out_d = nc.dram_tensor("out", [S, D], BF, kind="ExternalOutput").ap()

    with tile.TileContext(nc) as tc:
        import contextlib

        with contextlib.ExitStack() as ctx:
            # ---------- persistent tensors + constants ----------
            persist = ctx.enter_context(tc.tile_pool(name="persist", bufs=1))
            consts = ctx.enter_context(tc.tile_pool(name="consts", bufs=1))

            qhT_sb = persist.tile([128, 8, S], BF)   # [dk(+64 zero), head, sq]
            khT_sb = persist.tile([128, 8, SP], BF)  # [dk(+64 zero), head, sk]
            nc.gpsimd.memset(khT_sb[64:128, :, :], 0.0)
            nc.gpsimd.memset(qhT_sb[64:128, :, :], 0.0)
            vh_sb = persist.tile([128, NSK, 8, DK + 1], BF)  # ones col at 64
            concatT_sb = persist.tile([128, 4, S], BF)
            wo_sb = persist.tile([128, 4, D], BF)
            wq_sb = persist.tile([128, NJ, CPG], BF)
            qT1_sb = persist.tile([128, NJ, 1024], BF)  # q cols 1024:2048

            onesv_sb = consts.tile([128, NSK * 8], BF)
            nc.sync.dma_start(out=onesv_sb, in_=ones_d[:, 0 : NSK * 8])
            nc.scalar.copy(
                out=vh_sb[:, :, :, DK],
                in_=onesv_sb.rearrange("p (a b) -> p a b", a=NSK),
            )
            mb_sb = consts.tile([128, NSK], f32)
            nc.sync.dma_start(out=mb_sb, in_=mb_d)
            bqT_sb = consts.tile([128, 4], f32)
            nc.sync.dma_start(out=bqT_sb, in_=bqT_d)
            bkT_sb = consts.tile([128, 4], f32)
            nc.sync.dma_start(out=bkT_sb, in_=bkT_d)


            # column chunking of the khT free dim (PSUM banks are 512 f32)
            KCOLS = [(0, 512), (512, 512), (1024, 128)]

            # ---------- phase A (upfront): khT, vh, q block 0 ----------
            with contextlib.ExitStack() as actx:
                astage = actx.enter_context(tc.tile_pool(name="astage", bufs=1))
                prpool = actx.enter_context(
                    tc.tile_pool(name="prpool", bufs=8, space="PSUM")
                )

                kT_sb = astage.tile([128, NJ, SP], BF)
                vT_sb = astage.tile([128, NJ, SP], BF)
                qT0_sb = astage.tile([128, NJ, 1024], BF)
                wk_sb = astage.tile([128, NJ, CPG], BF)
                wv_sb = astage.tile([128, NJ, CPG], BF)
                nc.sync.dma_start(
                    out=wk_sb, in_=wk_d.rearrange("(j p) c -> p j c", p=128)
                )
                for jj in range(4):
                    nc.sync.dma_start(
                        out=kT_sb[:, 2 * jj : 2 * jj + 2, :],
                        in_=kT_d[256 * jj : 256 * jj + 256, :].rearrange(
                            "(j p) s -> p j s", p=128
                        ),
                    )
                nc.sync.dma_start(
                    out=wv_sb, in_=wv_d.rearrange("(j p) c -> p j c", p=128)
                )
                nc.sync.dma_start(
                    out=vT_sb[:, 0:4, :],
                    in_=vT_d[0:512, :].rearrange("(j p) s -> p j s", p=128),
                )
                nc.sync.dma_start(
                    out=vT_sb[:, 4:8, :],
                    in_=vT_d[512:1024, :].rearrange("(j p) s -> p j s", p=128),
                )
                nc.sync.dma_start(
                    out=qT0_sb,
                    in_=qT_d[:, 0:1024].rearrange("(j p) s -> p j s", p=128),
                )
                nc.sync.dma_start(
                    out=wq_sb, in_=wq_d.rearrange("(j p) c -> p j c", p=128)
                )
                nc.sync.dma_start(
                    out=qT1_sb,
                    in_=qT_d[:, 1024:2048].rearrange("(j p) s -> p j s", p=128),
                )
                nc.sync.dma_start(
                    out=wo_sb, in_=wo_d.rearrange("(j p) c -> p j c", p=128)
                )

                # khT[c, sk]: lhsT = wk chunk (stationary), rhs = kT stream
                for cch in range(4):
                    for c0, cw in KCOLS:
                        pr = prpool.tile([128, 512], f32, tag="pr", name="pr")
                        for j in range(NJ):
                            nc.tensor.matmul(
                                pr[:, 0:cw],
                                lhsT=wk_sb[:, j, cch * 128 : cch * 128 + 128],
                                rhs=kT_sb[:, j, c0 : c0 + cw],
                                start=(j == 0),
                                stop=(j == NJ - 1),
                            )
                        for hh in range(2):
                            nc.scalar.add(
                                khT_sb[0:64, 2 * cch + hh, c0 : c0 + cw],
                                pr[hh * 64 : hh * 64 + 64, 0:cw],
                                bkT_sb[hh * 64 : hh * 64 + 64, cch : cch + 1],
                            )

                # vh[sk, c] (+ones col): lhsT = vT chunk, rhs = wv
                for sk in range(NSK):
                    pr = prpool.tile([128, 512], f32, tag="pr", name="pr")
                    for j in range(NJ):
                        nc.tensor.matmul(
                            pr,
                            lhsT=vT_sb[:, j, sk * 128 : sk * 128 + 128],
                            rhs=wv_sb[:, j, :],
                            start=(j == 0),
                            stop=(j == NJ - 1),
                        )
                    nc.scalar.copy(
                        out=vh_sb[:, sk, :, 0:DK],
                        in_=pr.rearrange("p (h d) -> p h d", h=8),
                    )

                # qhT[c, sq] block 0 (sq 0:1024)
                for cch in range(4):
                    for cc in range(2):
                        c0 = cc * 512
                        pr = prpool.tile([128, 512], f32, tag="pr", name="pr")
                        for j in range(NJ):
                            nc.tensor.matmul(
                                pr,
                                lhsT=wq_sb[:, j, cch * 128 : cch * 128 + 128],
                                rhs=qT0_sb[:, j, c0 : c0 + 512],
                                start=(j == 0),
                                stop=(j == NJ - 1),
                            )
                        for hh in range(2):
                            nc.scalar.add(
                                qhT_sb[0:64, 2 * cch + hh, c0 : c0 + 512],
                                pr[hh * 64 : hh * 64 + 64, :],
                                bqT_sb[hh * 64 : hh * 64 + 64, cch : cch + 1],
                            )

            # ---------- phase B + fillers ----------
            with contextlib.ExitStack() as bctx:
                scpool = bctx.enter_context(
                    tc.tile_pool(name="scpool", bufs=2, space="PSUM")
                )
                atpool = bctx.enter_context(
                    tc.tile_pool(name="atpool", bufs=4, space="PSUM")
                )
                probpool = bctx.enter_context(tc.tile_pool(name="probpool", bufs=11))
                rcpool = bctx.enter_context(tc.tile_pool(name="rcpool", bufs=3))
                rcbpool = bctx.enter_context(tc.tile_pool(name="rcbpool", bufs=3))
                osbpool = bctx.enter_context(tc.tile_pool(name="osbpool", bufs=6))

                def finish_at(at_ps, sq2, h, half):
                    """attn PSUM (65x512, denom in row 64) -> concatT slice."""
                    pair, base = h // 2, (h % 2) * 64
                    rc = rcpool.tile([1, 512], f32, tag="rc", name="rc")
                    nc.vector.reciprocal(rc, at_ps[64:65, :])
                    rcb = rcbpool.tile([64, 512], f32, tag="rcb", name="rcb")
                    nc.gpsimd.partition_broadcast(rcb, rc)
                    q0 = sq2 * 1024 + half * 512
                    nc.vector.tensor_mul(
                        concatT_sb[base : base + 64, pair, q0 : q0 + 512],
                        at_ps[0:64, :],
                        rcb,
                    )

                # ----- filler units -----
                def q_unit(cch, cc):
                    def emit():
                        c0 = cc * 512
                        pr = atpool.tile([128, 512], f32, tag="at", name="qpr")
                        for j in range(NJ):
                            nc.tensor.matmul(
                                pr,
                                lhsT=wq_sb[:, j, cch * 128 : cch * 128 + 128],
                                rhs=qT1_sb[:, j, c0 : c0 + 512],
                                start=(j == 0),
                                stop=(j == NJ - 1),
                            )
                        for hh in range(2):
                            nc.vector.tensor_scalar_add(
                                qhT_sb[0:64, 2 * cch + hh, 1024 + c0 : 1536 + c0],
                                pr[hh * 64 : hh * 64 + 64, :],
                                bqT_sb[hh * 64 : hh * 64 + 64, cch : cch + 1],
                            )

                    return emit

                def emit_c(sqc, do, pool, tag, width):
                    o_ps = pool.tile([128, width], f32, tag=tag, name="ops")
                    ops = o_ps[:, 0:512]
                    for j in range(4):
                        nc.tensor.matmul(
                            ops,
                            lhsT=concatT_sb[:, j, sqc * 128 : sqc * 128 + 128],
                            rhs=wo_sb[:, j, do * 512 : do * 512 + 512],
                            start=(j == 0),
                            stop=(j == 3),
                        )
                    o_sb = osbpool.tile([128, 512], BF, tag="osb", name="osb")
                    nc.vector.tensor_copy(out=o_sb, in_=ops)
                    nc.sync.dma_start(
                        out=out_d[
                            sqc * 128 : sqc * 128 + 128,
                            do * 512 : do * 512 + 512,
                        ],
                        in_=o_sb,
                    )

                def c_unit(sqc, do):
                    return lambda: emit_c(sqc, do, atpool, "at", 512)

                fillers = {}
                qunits = [q_unit(cch, cc) for cch in range(4) for cc in range(2)]
                for i, u in enumerate(qunits):
                    fillers.setdefault(i + 1, []).append(u)  # slots 1..8
                cpend = [(sqc, do) for sqc in range(8) for do in range(2)]
                slot_of = [9, 9, 9, 10, 10, 10, 11, 11, 12, 12, 13, 13, 14, 14, 15, 15]
                for i, (sqc, do) in enumerate(cpend):
                    fillers.setdefault(slot_of[i], []).append(c_unit(sqc, do))

                # ----- main software-pipelined slot loop -----
                slots = [(sq2, h) for sq2 in (0, 1) for h in range(8)]
                prev = None  # (at1_ps emitted?, probs list, sq2, h)

                for i, (sq2, h) in enumerate(slots):
                    pair, base = h // 2, (h % 2) * 64
                    qoff = sq2 * 1024
                    for u in fillers.get(i, ()):
                        u()
                    at0 = atpool.tile([128, 512], f32, tag="at", name="at0")
                    at1_prev = atpool.tile([128, 512], f32, tag="at", name="at1") if prev else None
                    probs_list = []
                    for sk in range(NSK):
                        sc = scpool.tile([128, 1024], f32, tag="sc")
                        for half in range(2):
                            nc.tensor.matmul(
                                sc[:, half * 512 : half * 512 + 512],
                                lhsT=khT_sb[:, h, sk * 128 : sk * 128 + 128],
                                rhs=qhT_sb[
                                    :, h,
                                    qoff + half * 512 : qoff + half * 512 + 512,
                                ],
                                start=True,
                                stop=True,
                            )
                        # interleave: attn-half1 of the previous slot,
                        # shifted one step so its first matmul never waits on
                        # the just-freed at bank
                        if prev and sk >= 1:
                            psq2, ph, pprobs = prev
                            nc.tensor.matmul(
                                at1_prev[0:65, :],
                                lhsT=vh_sb[:, sk - 1, ph, :],
                                rhs=pprobs[sk - 1][:, 512:1024],
                                start=(sk == 1),
                                stop=False,
                            )
                        # attn-half0 of this slot, one step delayed
                        if sk > 0:
                            nc.tensor.matmul(
                                at0[0:65, :],
                                lhsT=vh_sb[:, sk - 1, h, :],
                                rhs=probs_list[sk - 1][:, 0:512],
                                start=(sk - 1 == 0),
                                stop=False,
                            )
                        probs = probpool.tile([128, 1024], BF, tag="probs", name="probs")
                        probs_list.append(probs)
                        nc.scalar.activation(
                            out=probs,
                            in_=sc,
                            func=Exp,
                            bias=mb_sb[:, sk : sk + 1],
                            scale=0.125,
                        )
                    if prev:
                        nc.tensor.matmul(
                            at1_prev[0:65, :],
                            lhsT=vh_sb[:, NSK - 1, prev[1], :],
                            rhs=prev[2][NSK - 1][:, 512:1024],
                            start=False,
                            stop=True,
                        )
                        finish_at(at1_prev, prev[0], prev[1], 1)
                    nc.tensor.matmul(
                        at0[0:65, :],
                        lhsT=vh_sb[:, NSK - 1, h, :],
                        rhs=probs_list[NSK - 1][:, 0:512],
                        start=False,
                        stop=True,
                    )
                    finish_at(at0, sq2, h, 0)
                    prev = (sq2, h, probs_list)

                # drain: attn-half1 of the last slot, interleaved with the
                # out-projection units that only need half0 norms (query rows
                # 1024..1535 = sqc 8..11, finished by the end of slot 15)
                psq2, ph, pprobs = prev
                at1_last = atpool.tile([128, 512], f32, tag="at", name="at1l")
                half0_units = [(8 + sqc, do) for sqc in range(4) for do in range(2)]
                half1_units = [(12 + sqc, do) for sqc in range(4) for do in range(2)]
                rings = [
                    (scpool, "sc", 1024),
                    (atpool, "at", 512),
                    (scpool, "sc", 1024),
                ]
                t = 0
                for sk in range(NSK):
                    nc.tensor.matmul(
                        at1_last[0:65, :],
                        lhsT=vh_sb[:, sk, ph, :],
                        rhs=pprobs[sk][:, 512:1024],
                        start=(sk == 0),
                        stop=(sk == NSK - 1),
                    )
                    if sk % 2 == 1 and half0_units:
                        sqc, do = half0_units.pop(0)
                        pool, tag, width = rings[t % 3]
                        t += 1
                        emit_c(sqc, do, pool, tag, width)
                finish_at(at1_last, psq2, ph, 1)
                for sqc, do in half0_units + half1_units:
                    pool, tag, width = rings[t % 3]
                    t += 1
                    emit_c(sqc, do, pool, tag, width)

    nc.compile()
    return nc


def get_nc():
    if "nc" not in _cache:
        _cache["nc"] = _build_nc()
    return _cache["nc"]


def make_in_maps(q, k, v, mask, Wq, bq, Wk, bk, Wv, bv, Wo, bo):
    import ml_dtypes

    f32 = np.float32
    bf16 = ml_dtypes.bfloat16
    c = np.ascontiguousarray
    in_maps = []
    for core in range(NCORES):
        b, g = core // 2, core % 2
        cols = slice(g * CPG, (g + 1) * CPG)
        m = np.asarray(mask[b, 0])
        idx = np.flatnonzero(m)
        ns = len(idx)
        assert ns <= SP, f"batch {b}: {ns} unmasked keys > SP={SP}"
        idx_pad = np.concatenate([idx, np.zeros(SP - ns, np.int64)])
        mb = np.zeros((128, NSK), f32)
        flat = np.arange(SP) >= ns
        mb[flat.reshape(NSK, 128).T] = -1e9
        qT = np.asarray(q[b], f32).T
        kT = np.asarray(k[b], f32).T[:, idx_pad]
        vT = np.asarray(v[b], f32).T[:, idx_pad]
        in_maps.append(
            {
                "qT": c(qT.astype(bf16)),
                "kT": c(kT.astype(bf16)),
                "vT": c(vT.astype(bf16)),
                "wq": c(np.asarray(Wq[:, cols], f32).astype(bf16)),
                "wk": c(np.asarray(Wk[:, cols], f32).astype(bf16)),
                "wv": c(np.asarray(Wv[:, cols], f32).astype(bf16)),
                "wo": c(np.asarray(Wo[cols, :], f32).astype(bf16)),
                "bqT": c(np.asarray(bq[cols], f32).reshape(4, 128).T),
                "bkT": c(np.asarray(bk[cols], f32).reshape(4, 128).T),
                "maskbias": mb,
                "ones": np.ones((128, 512), bf16),
            }
        )
    return in_maps


def gather(results, bo, bv_wo=None):
    out = np.zeros((B, S, D), np.float32)
    for core in range(NCORES):
        b = core // 2
        out[b] += np.asarray(results[core]["out"], np.float32)
    bias = np.asarray(bo, np.float64)
    if bv_wo is not None:
        bias = bias + bv_wo
    out += bias.astype(np.float32)[None, None, :]
    return out


def run_on_hw(in_maps, trace=False, trace_cores=None):
    from concourse.bass_utils import run_bass_kernel_spmd

    nc = get_nc()
    return run_bass_kernel_spmd(
        nc,
        in_maps,
        list(range(NCORES)),
        trace=trace,
        trace_cores=trace_cores,
    )


def kernel(q, k, v, mask, Wq, bq, Wk, bk, Wv, bv, Wo, bo):
    in_maps = make_in_maps(q, k, v, mask, Wq, bq, Wk, bk, Wv, bv, Wo, bo)
    res = run_on_hw(in_maps)
    bv_wo = np.asarray(bv, np.float64) @ np.asarray(Wo, np.float64)
    return gather(res.results, bo, bv_wo)


# revision 29
# speedup vs baseline: 1.1805x; 1.1805x over previous
"""Multi-head attention (B=4, S=2048, D=1024, H=16, Dk=64) on 8 trn2 NeuronCores.

Sharding: core = (batch b, head-group g), b in 0..3, g in 0..1.  Each core
computes attention for its batch and its 8 heads plus the partial out
projection for its 512 rows of Wo; host sums the two partials per batch and
adds bo.

Key structural ideas (vs the 922us baseline):
  * Host-side prep: q/k/v are transposed to [D, S] on the host, so the device
    does no PE transposes at all.  k/v are additionally COMPACTED: only the
    ~1024 unmasked key rows (mask==1) are sent, padded to SP=1152 (9 chunks of
    128 instead of 16) - the mask zeroes ~44% of the attention work, so skip
    it.  Padding slots get a -1e9 bias in the exp (probs underflow to 0).
  * bf16 everywhere on the PE (f32r runs ~1.5ns/row with 300ns serial weight
    loads on real hw; bf16 streams 1 col/cycle with hidden loads); PSUM
    accumulates f32.  khT/qhT are stored per-head as [128, head, s] with rows
    64..127 zeroed so every matmul in the program uses the same 128x128 PE
    tile configuration - mixing tile configs serializes the PE pipeline.
  * No K=1 bias matmuls: q/k biases ride along the PSUM->SBUF copies
    (per-partition bias on ACT/DVE); the v bias is folded out exactly
    (softmax rows sum to 1, so bv shifts attn by bv: the host adds bv @ Wo
    to the output bias instead).
  * Phase B is software-pipelined per 1024-query block and head: per sk step
    the PE emission order is [scores(sk) x2, attn-half1(prev head, sk),
    attn-half0(sk)] so the PE never sits behind the ACT exp (the pacer at
    ~1.1us per 128x1024 exp); the attn matmul is split into two 512-wide
    halves accumulated in different passes so the at tiles are single-bank,
    leaving one spare PSUM bank for filler work.
  * Filler work keeps the PE busy through B's exp-paced stretch: the q
    projection for the second query block runs inside phase B of the first
    block, and the out projection of block 0 runs inside phase B of block 1.
  * Few, large, strided input DMAs (one per tensor): many small dma_starts
    push Tile's semaphore-wrap barriers into the middle of the input stream,
    which can serialize the whole kernel behind phase-B progress.
  * Softmax denominator comes free from a ones column appended to vh (row 64
    of the attn PSUM); normalization = DVE reciprocal + gpsimd
    partition_broadcast + DVE multiply (no PE replicate matmuls, no PSUM).
"""

import sys

sys.path.insert(0, "/opt/trn_rl_repo")

import numpy as np

B, S, D, H, DK = 4, 2048, 1024, 16, 64
CPG = 512          # projection columns per core (8 heads x 64)
SP = 1152          # compacted+padded key count (9 chunks of 128)
NSK = SP // 128    # 9
NJ = D // 128      # 8 contraction chunks
NCORES = 8

_cache = {}


def _build_nc():
    import concourse.bass as bass
    import concourse.tile as tile
    from concourse import bacc, mybir

    f32 = mybir.dt.float32
    R = mybir.dt.float32r
    BF = mybir.dt.bfloat16
    Exp = mybir.ActivationFunctionType.Exp

    nc = bacc.Bacc("TRN2", target_bir_lowering=False, debug=False)

    qT_d = nc.dram_tensor("qT", [D, S], BF, kind="ExternalInput").ap()
    kT_d = nc.dram_tensor("kT", [D, SP], BF, kind="ExternalInput").ap()
    vT_d = nc.dram_tensor("vT", [D, SP], BF, kind="ExternalInput").ap()
    wq_d = nc.dram_tensor("wq", [D, CPG], BF, kind="ExternalInput").ap()
    wk_d = nc.dram_tensor("wk", [D, CPG], BF, kind="ExternalInput").ap()
    wv_d = nc.dram_tensor("wv", [D, CPG], BF, kind="ExternalInput").ap()
    wo_d = nc.dram_tensor("wo", [CPG, D], BF, kind="ExternalInput").ap()
    bqT_d = nc.dram_tensor("bqT", [128, 4], f32, kind="ExternalInput").ap()
    bkT_d = nc.dram_tensor("bkT", [128, 4], f32, kind="ExternalInput").ap()
    mb_d = nc.dram_tensor("maskbias", [128, NSK], f32, kind="ExternalInput").ap()
    ones_d = nc.dram_tensor("ones", [128, 512], BF, kind="ExternalInput").ap()
    out_d = nc.dram_tensor("out", [S, D], BF, kind="ExternalOutput").ap()

    with tile.TileContext(nc) as tc:
        import contextlib

        with contextlib.ExitStack() as ctx:
            # ---------- persistent tensors + constants ----------
            persist = ctx.enter_context(tc.tile_pool(name="persist", bufs=1))
            consts = ctx.enter_context(tc.tile_pool(name="consts", bufs=1))

            qhT_sb = persist.tile([128, 8, S], BF)   # [dk(+64 zero), head, sq]
            khT_sb = persist.tile([128, 8, SP], BF)  # [dk(+64 zero), head, sk]
            nc.gpsimd.memset(khT_sb[64:128, :, :], 0.0)
            nc.gpsimd.memset(qhT_sb[64:128, :, :], 0.0)
            vh_sb = persist.tile([128, NSK, 8, DK + 1], BF)  # ones col at 64
            concatT_sb = persist.tile([128, 4, S], BF)
            wo_sb = persist.tile([128, 4, D], BF)
            wq_sb = persist.tile([128, NJ, CPG], BF)
            qT1_sb = persist.tile([128, NJ, 1024], BF)  # q cols 1024:2048

            onesv_sb = consts.tile([128, NSK * 8], BF)
            nc.sync.dma_start(out=onesv_sb, in_=ones_d[:, 0 : NSK * 8])
            nc.scalar.copy(
                out=vh_sb[:, :, :, DK],
                in_=onesv_sb.rearrange("p (a b) -> p a b", a=NSK),
            )
            mb_sb = consts.tile([128, NSK], f32)
            nc.sync.dma_start(out=mb_sb, in_=mb_d)
            bqT_sb = consts.tile([128, 4], f32)
            nc.sync.dma_start(out=bqT_sb, in_=bqT_d)
            bkT_sb = consts.tile([128, 4], f32)
            nc.sync.dma_start(out=bkT_sb, in_=bkT_d)


            # column chunking of the khT free dim (PSUM banks are 512 f32)
            KCOLS = [(0, 512), (512, 512), (1024, 128)]

            # ---------- phase A (upfront): khT, vh, q block 0 ----------
            with contextlib.ExitStack() as actx:
                astage = actx.enter_context(tc.tile_pool(name="astage", bufs=1))
                prpool = actx.enter_context(
                    tc.tile_pool(name="prpool", bufs=8, space="PSUM")
                )

                kT_sb = astage.tile([128, NJ, SP], BF)
                vT_sb = astage.tile([128, NJ, SP], BF)
                qT0_sb = astage.tile([128, NJ, 1024], BF)
                wk_sb = astage.tile([128, NJ, CPG], BF)
                wv_sb = astage.tile([128, NJ, CPG], BF)
                nc.sync.dma_start(
                    out=wk_sb, in_=wk_d.rearrange("(j p) c -> p j c", p=128)
                )
                nc.sync.dma_start(
                    out=kT_sb[:, 0:4, :],
                    in_=kT_d[0:512, :].rearrange("(j p) s -> p j s", p=128),
                )
                nc.sync.dma_start(
                    out=kT_sb[:, 4:8, :],
                    in_=kT_d[512:1024, :].rearrange("(j p) s -> p j s", p=128),
                )
                nc.sync.dma_start(
                    out=wv_sb, in_=wv_d.rearrange("(j p) c -> p j c", p=128)
                )
                nc.sync.dma_start(
                    out=vT_sb[:, 0:4, :],
                    in_=vT_d[0:512, :].rearrange("(j p) s -> p j s", p=128),
                )
                nc.sync.dma_start(
                    out=vT_sb[:, 4:8, :],
                    in_=vT_d[512:1024, :].rearrange("(j p) s -> p j s", p=128),
                )
                nc.sync.dma_start(
                    out=qT0_sb,
                    in_=qT_d[:, 0:1024].rearrange("(j p) s -> p j s", p=128),
                )
                nc.sync.dma_start(
                    out=wq_sb, in_=wq_d.rearrange("(j p) c -> p j c", p=128)
                )
                nc.sync.dma_start(
                    out=qT1_sb,
                    in_=qT_d[:, 1024:2048].rearrange("(j p) s -> p j s", p=128),
                )
                nc.sync.dma_start(
                    out=wo_sb, in_=wo_d.rearrange("(j p) c -> p j c", p=128)
                )

                # khT[c, sk]: lhsT = wk chunk (stationary), rhs = kT stream
                for cch in range(4):
                    for c0, cw in KCOLS:
                        pr = prpool.tile([128, 512], f32, tag="pr", name="pr")
                        for j in range(NJ):
                            nc.tensor.matmul(
                                pr[:, 0:cw],
                                lhsT=wk_sb[:, j, cch * 128 : cch * 128 + 128],
                                rhs=kT_sb[:, j, c0 : c0 + cw],
                                start=(j == 0),
                                stop=(j == NJ - 1),
                            )
                        for hh in range(2):
                            nc.scalar.add(
                                khT_sb[0:64, 2 * cch + hh, c0 : c0 + cw],
                                pr[hh * 64 : hh * 64 + 64, 0:cw],
                                bkT_sb[hh * 64 : hh * 64 + 64, cch : cch + 1],
                            )

                # vh[sk, c] (+ones col): lhsT = vT chunk, rhs = wv
                for sk in range(NSK):
                    pr = prpool.tile([128, 512], f32, tag="pr", name="pr")
                    for j in range(NJ):
                        nc.tensor.matmul(
                            pr,
                            lhsT=vT_sb[:, j, sk * 128 : sk * 128 + 128],
                            rhs=wv_sb[:, j, :],
                            start=(j == 0),
                            stop=(j == NJ - 1),
                        )
                    nc.scalar.copy(
                        out=vh_sb[:, sk, :, 0:DK],
                        in_=pr.rearrange("p (h d) -> p h d", h=8),
                    )

                # qhT[c, sq] block 0 (sq 0:1024)
                for cch in range(4):
                    for cc in range(2):
                        c0 = cc * 512
                        pr = prpool.tile([128, 512], f32, tag="pr", name="pr")
                        for j in range(NJ):
                            nc.tensor.matmul(
                                pr,
                                lhsT=wq_sb[:, j, cch * 128 : cch * 128 + 128],
                                rhs=qT0_sb[:, j, c0 : c0 + 512],
                                start=(j == 0),
                                stop=(j == NJ - 1),
                            )
                        for hh in range(2):
                            nc.scalar.add(
                                qhT_sb[0:64, 2 * cch + hh, c0 : c0 + 512],
                                pr[hh * 64 : hh * 64 + 64, :],
                                bqT_sb[hh * 64 : hh * 64 + 64, cch : cch + 1],
                            )

            # ---------- phase B + fillers ----------
            with contextlib.ExitStack() as bctx:
                scpool = bctx.enter_context(
                    tc.tile_pool(name="scpool", bufs=2, space="PSUM")
                )
                atpool = bctx.enter_context(
                    tc.tile_pool(name="atpool", bufs=4, space="PSUM")
                )
                probpool = bctx.enter_context(tc.tile_pool(name="probpool", bufs=13))
                rcpool = bctx.enter_context(tc.tile_pool(name="rcpool", bufs=3))
                rcbpool = bctx.enter_context(tc.tile_pool(name="rcbpool", bufs=3))
                osbpool = bctx.enter_context(tc.tile_pool(name="osbpool", bufs=6))

                def finish_at(at_ps, sq2, h, half):
                    """attn PSUM (65x512, denom in row 64) -> concatT slice."""
                    pair, base = h // 2, (h % 2) * 64
                    rc = rcpool.tile([1, 512], f32, tag="rc", name="rc")
                    nc.vector.reciprocal(rc, at_ps[64:65, :])
                    rcb = rcbpool.tile([64, 512], f32, tag="rcb", name="rcb")
                    nc.gpsimd.partition_broadcast(rcb, rc)
                    q0 = sq2 * 1024 + half * 512
                    nc.vector.tensor_mul(
                        concatT_sb[base : base + 64, pair, q0 : q0 + 512],
                        at_ps[0:64, :],
                        rcb,
                    )

                # ----- filler units -----
                def q_unit(cch, cc):
                    def emit():
                        c0 = cc * 512
                        pr = atpool.tile([128, 512], f32, tag="at", name="qpr")
                        for j in range(NJ):
                            nc.tensor.matmul(
                                pr,
                                lhsT=wq_sb[:, j, cch * 128 : cch * 128 + 128],
                                rhs=qT1_sb[:, j, c0 : c0 + 512],
                                start=(j == 0),
                                stop=(j == NJ - 1),
                            )
                        for hh in range(2):
                            nc.vector.tensor_scalar_add(
                                qhT_sb[0:64, 2 * cch + hh, 1024 + c0 : 1536 + c0],
                                pr[hh * 64 : hh * 64 + 64, :],
                                bqT_sb[hh * 64 : hh * 64 + 64, cch : cch + 1],
                            )

                    return emit

                def emit_c(sqc, do, pool, tag, width):
                    o_ps = pool.tile([128, width], f32, tag=tag, name="ops")
                    ops = o_ps[:, 0:512]
                    for j in range(4):
                        nc.tensor.matmul(
                            ops,
                            lhsT=concatT_sb[:, j, sqc * 128 : sqc * 128 + 128],
                            rhs=wo_sb[:, j, do * 512 : do * 512 + 512],
                            start=(j == 0),
                            stop=(j == 3),
                        )
                    o_sb = osbpool.tile([128, 512], BF, tag="osb", name="osb")
                    nc.vector.tensor_copy(out=o_sb, in_=ops)
                    nc.sync.dma_start(
                        out=out_d[
                            sqc * 128 : sqc * 128 + 128,
                            do * 512 : do * 512 + 512,
                        ],
                        in_=o_sb,
                    )

                def c_unit(sqc, do):
                    return lambda: emit_c(sqc, do, atpool, "at", 512)

                fillers = {}
                qunits = [q_unit(cch, cc) for cch in range(4) for cc in range(2)]
                for i, u in enumerate(qunits):
                    fillers.setdefault(i, []).append(u)  # slots 0..7
                cpend = [(sqc, do) for sqc in range(8) for do in range(2)]
                slot_of = [9, 9, 9, 10, 10, 10, 11, 11, 12, 12, 13, 13, 14, 14, 15, 15]
                for i, (sqc, do) in enumerate(cpend):
                    fillers.setdefault(slot_of[i], []).append(c_unit(sqc, do))

                # ----- main software-pipelined slot loop -----
                slots = [(sq2, h) for sq2 in (0, 1) for h in range(8)]
                prev = None  # (at1_ps emitted?, probs list, sq2, h)

                for i, (sq2, h) in enumerate(slots):
                    pair, base = h // 2, (h % 2) * 64
                    qoff = sq2 * 1024
                    for u in fillers.get(i, ()):
                        u()
                    at0 = atpool.tile([128, 512], f32, tag="at", name="at0")
                    at1_prev = atpool.tile([128, 512], f32, tag="at", name="at1") if prev else None
                    probs_list = []
                    for sk in range(NSK):
                        sc = scpool.tile([128, 1024], f32, tag="sc")
                        for half in range(2):
                            nc.tensor.matmul(
                                sc[:, half * 512 : half * 512 + 512],
                                lhsT=khT_sb[:, h, sk * 128 : sk * 128 + 128],
                                rhs=qhT_sb[
                                    :, h,
                                    qoff + half * 512 : qoff + half * 512 + 512,
                                ],
                                start=True,
                                stop=True,
                            )
                        # interleave: attn-half1 of the previous slot,
                        # shifted one step so its first matmul never waits on
                        # the just-freed at bank
                        if prev and sk >= 1:
                            psq2, ph, pprobs = prev
                            nc.tensor.matmul(
                                at1_prev[0:65, :],
                                lhsT=vh_sb[:, sk - 1, ph, :],
                                rhs=pprobs[sk - 1][:, 512:1024],
                                start=(sk == 1),
                                stop=False,
                            )
                        # attn-half0 of this slot, one step delayed
                        if sk > 0:
                            nc.tensor.matmul(
                                at0[0:65, :],
                                lhsT=vh_sb[:, sk - 1, h, :],
                                rhs=probs_list[sk - 1][:, 0:512],
                                start=(sk - 1 == 0),
                                stop=False,
                            )
                        probs = probpool.tile([128, 1024], BF, tag="probs", name="probs")
                        probs_list.append(probs)
                        nc.scalar.activation(
                            out=probs,
                            in_=sc,
                            func=Exp,
                            bias=mb_sb[:, sk : sk + 1],
                            scale=0.125,
                        )
                    if prev:
                        nc.tensor.matmul(
                            at1_prev[0:65, :],
                            lhsT=vh_sb[:, NSK - 1, prev[1], :],
                            rhs=prev[2][NSK - 1][:, 512:1024],
                            start=False,
                            stop=True,
                        )
                        finish_at(at1_prev, prev[0], prev[1], 1)
                    nc.tensor.matmul(
                        at0[0:65, :],
                        lhsT=vh_sb[:, NSK - 1, h, :],
                        rhs=probs_list[NSK - 1][:, 0:512],
                        start=False,
                        stop=True,
                    )
                    finish_at(at0, sq2, h, 0)
                    prev = (sq2, h, probs_list)

                # drain: attn-half1 of the last slot, interleaved with the
                # out-projection units that only need half0 norms (query rows
                # 1024..1535 = sqc 8..11, finished by the end of slot 15)
                psq2, ph, pprobs = prev
                at1_last = atpool.tile([128, 512], f32, tag="at", name="at1l")
                half0_units = [(8 + sqc, do) for sqc in range(4) for do in range(2)]
                half1_units = [(12 + sqc, do) for sqc in range(4) for do in range(2)]
                rings = [
                    (scpool, "sc", 1024),
                    (atpool, "at", 512),
                    (scpool, "sc", 1024),
                ]
                t = 0
                for sk in range(NSK):
                    nc.tensor.matmul(
                        at1_last[0:65, :],
                        lhsT=vh_sb[:, sk, ph, :],
                        rhs=pprobs[sk][:, 512:1024],
                        start=(sk == 0),
                        stop=(sk == NSK - 1),
                    )
                    if sk % 2 == 1 and half0_units:
                        sqc, do = half0_units.pop(0)
                        pool, tag, width = rings[t % 3]
                        t += 1
                        emit_c(sqc, do, pool, tag, width)
                finish_at(at1_last, psq2, ph, 1)
                for sqc, do in half0_units + half1_units:
                    pool, tag, width = rings[t % 3]
                    t += 1
                    emit_c(sqc, do, pool, tag, width)

    nc.compile()
    return nc


def get_nc():
    if "nc" not in _cache:
        _cache["nc"] = _build_nc()
    return _cache["nc"]


def make_in_maps(q, k, v, mask, Wq, bq, Wk, bk, Wv, bv, Wo, bo):
    import ml_dtypes

    f32 = np.float32
    bf16 = ml_dtypes.bfloat16
    c = np.ascontiguousarray
    in_maps = []
    for core in range(NCORES):
        b, g = core // 2, core % 2
        cols = slice(g * CPG, (g + 1) * CPG)
        m = np.asarray(mask[b, 0])
        idx = np.flatnonzero(m)
        ns = len(idx)
        assert ns <= SP, f"batch {b}: {ns} unmasked keys > SP={SP}"
        idx_pad = np.concatenate([idx, np.zeros(SP - ns, np.int64)])
        mb = np.zeros((128, NSK), f32)
        flat = np.arange(SP) >= ns
        mb[flat.reshape(NSK, 128).T] = -1e9
        qT = np.asarray(q[b], f32).T
        kT = np.asarray(k[b], f32).T[:, idx_pad]
        vT = np.asarray(v[b], f32).T[:, idx_pad]
        in_maps.append(
            {
                "qT": c(qT.astype(bf16)),
                "kT": c(kT.astype(bf16)),
                "vT": c(vT.astype(bf16)),
                "wq": c(np.asarray(Wq[:, cols], f32).astype(bf16)),
                "wk": c(np.asarray(Wk[:, cols], f32).astype(bf16)),
                "wv": c(np.asarray(Wv[:, cols], f32).astype(bf16)),
                "wo": c(np.asarray(Wo[cols, :], f32).astype(bf16)),
                "bqT": c(np.asarray(bq[cols], f32).reshape(4, 128).T),
                "bkT": c(np.asarray(bk[cols], f32).reshape(4, 128).T),
                "maskbias": mb,
                "ones": np.ones((128, 512), bf16),
            }
        )
    return in_maps


def gather(results, bo, bv_wo=None):
    out = np.zeros((B, S, D), np.float32)
    for core in range(NCORES):
        b = core // 2
        out[b] += np.asarray(results[core]["out"], np.float32)
    bias = np.asarray(bo, np.float64)
    if bv_wo is not None:
        bias = bias + bv_wo
    out += bias.astype(np.float32)[None, None, :]
    return out


def run_on_hw(in_maps, trace=False, trace_cores=None):
    from concourse.bass_utils import run_bass_kernel_spmd

    nc = get_nc()
    return run_bass_kernel_spmd(
        nc,
        in_maps,
        list(range(NCORES)),
        trace=trace,
        trace_cores=trace_cores,
    )


def kernel(q, k, v, mask, Wq, bq, Wk, bk, Wv, bv, Wo, bo):
    in_maps = make_in_maps(q, k, v, mask, Wq, bq, Wk, bk, Wv, bv, Wo, bo)
    res = run_on_hw(in_maps)
    bv_wo = np.asarray(bv, np.float64) @ np.asarray(Wo, np.float64)
    return gather(res.results, bo, bv_wo)
